# revision 46
# baseline (speedup 1.0000x reference)
"""Trainium2 Bass kernel: scatter rows of input_ into a zero-initialized
[output_size, D] bf16 buffer: out[indices[i], :] = input_[i, :] (last
occurrence wins for duplicate indices).

Strategy (8 NeuronCores), measured at ~330us HW vs the 426us fixed-class
baseline (both bit-exact):
  - Output row-sharded by index range: core k owns rows [k*SHARD, (k+1)*SHARD).
  - Each core's shard is split into 65280-row regions, each followed by a
    128-row scratch strip in the out tensor (idx is an int16 2-row-block
    number, so a region maxes out at ~65408 rows; the strip absorbs
    quantization slack and hosts pad-descriptor targets).
  - Host routing: dedup indices last-wins; cover the written rows with
    dma_scatter_add descriptors: maximal written-row runs are merged across
    gaps of <=GAP unwritten rows (bridged rows are zero-filled in the packed
    data; CCE add of 0 is a no-op), then split into pieces of 1..4 rows.
    Keeping every CCE packet <=512B matters: measured per-packet SDMA engine
    time is ~28.5ns + ~0.072ns/B up to 512B but grows superlinearly past
    that (640-768B packets cost ~0.1ns/B), so size classes 5+ lose.
  - An overlap-resolution pass shrinks/splits any piece whose quantization
    slack would reach the next piece: concurrent CCE read-modify-writes on
    shared bytes race across queues and lose updates.
  - Pieces are grouped into (size, start-parity) classes per region; scarce
    (class, region) groups are promoted upward (CONSOLIDATE) so calls don't
    drown in 128-rounding pads. Pad slots carry valid targets spread over
    the scratch strip with all-zero data (+0 races with other pads are
    benign). Trailing -1 pads would desync the decode-side ring bookkeeping,
    which sizes the ring from num_idxs_reg while the Q7 trims -> hang.
  - Packed data is loaded in ~5MB groups into a 4-buffer column ring (one
    dma_start per group; per-chunk loads fragment into ~1.5KB descriptors
    that pay ~30ns each). One load-completion semaphore PER BUFFER: a single
    shared counter is ambiguous under SDMA engine skew (15 engines can run a
    whole load ahead while one lags, satisfying the aggregate threshold
    while that load is incomplete on the straggler -> stale-data scatters).
  - Chunks are LPT-balanced across the 4 SWDGE queues (Q7 core pairs) by
    estimated descriptor-generation time and emitted round-robin so all 4
    queue pairs generate concurrently.
  - The output is donated pre-zeroed by run_bass_kernel_spmd / bass2jax, so
    CCE add == set (every written row is added exactly once onto zeros).
"""

import os
import sys

sys.path.insert(0, "/opt/trn_rl_repo")
os.environ.setdefault("JAX_PLATFORMS", "axon")

import numpy as np
import ml_dtypes

from concourse import bacc, mybir
from concourse.bass import AP
from concourse import bass_utils

N_CORES = 8
REGION_ROWS = 65280  # idx = 2-row block number: 32640 blocks, int16-safe
PAD_ROWS = 128  # scratch strip after each region: pad targets + slack spill
SLOT_ROWS = REGION_ROWS + PAD_ROWS  # region slot stride in the out tensor
GAP = 2  # merge written-row runs separated by <= GAP unwritten rows
MAXS = 8  # max descriptor size in rows
SIZES = (1, 2, 3, 4, 8)  # class sizes (rows)
CH_CAP = 7936  # per-call index cap: tx ring needs 2*CH/16+1 < 1024 descs
CAP_ELEMS = 786432  # per-call data tile cap (CH * E elems, 6144 SBUF cols)
NQ = 4  # SWDGE queues == Q7 core pairs generating descriptors in parallel
NBG = 4  # SBUF load-group ring buffers
GROUP_COLS = 20480  # columns per load-group buffer (40 KiB / partition)
CONSOLIDATE = 96  # re-quantize (class, region) groups smaller than this

D = 64
BSTEP = 2  # descriptor address step: 2 rows = 256B, idx in 2-row blocks


def _runs_of(mask):
    d = np.diff(np.concatenate([[0], mask.view(np.int8), [0]]))
    return np.flatnonzero(d == 1), np.flatnonzero(d == -1)  # starts, ends(excl)


def _cover_pieces(written, OUT, SHARD, n_region):
    """Global covering: returns (starts, sizes, qsizes) of descriptor pieces.
    Runs are split at shard-local region boundaries, gap-merged, split to
    <=MAXS, and remainders quantized up to the class set. Quantization slack
    may spill into the region's scratch strip, which is harmless."""

    def rid(g):
        cc = g // SHARD
        return cc * n_region + np.minimum((g - cc * SHARD) // REGION_ROWS,
                                          n_region - 1)

    starts, ends = _runs_of(written)
    # split runs crossing region boundaries
    cross = np.flatnonzero(rid(starts) != rid(ends - 1))
    if len(cross):
        add_s, add_e = [], []
        for i in cross:
            s, e = int(starts[i]), int(ends[i])
            cuts = set()
            for cc in range(s // SHARD, e // SHARD + 1):
                if s < cc * SHARD < e:
                    cuts.add(cc * SHARD)
                for r in range(1, n_region):
                    b = cc * SHARD + r * REGION_ROWS
                    if s < b < e:
                        cuts.add(b)
            seq = np.array([s] + sorted(cuts) + [e])
            add_s.append(seq[:-1])
            add_e.append(seq[1:])
        keep = np.ones(len(starts), bool)
        keep[cross] = False
        starts = np.concatenate([starts[keep]] + add_s)
        ends = np.concatenate([ends[keep]] + add_e)
        o = np.argsort(starts)
        starts, ends = starts[o], ends[o]
    # gap-merge within regions
    gap_ok = (starts[1:] - ends[:-1]) <= GAP
    same_reg = rid(starts[1:]) == rid(ends[:-1] - 1)
    brk = np.concatenate([[True], ~(gap_ok & same_reg)])
    gid = np.cumsum(brk) - 1
    ngroups = gid[-1] + 1
    m_start = np.full(ngroups, OUT, dtype=np.int64)
    np.minimum.at(m_start, gid, starts)
    m_end = np.zeros(ngroups, dtype=np.int64)
    np.maximum.at(m_end, gid, ends)
    L = m_end - m_start
    # split into pieces of MAXS + quantized remainder
    nfull = L // MAXS
    rem = L - nfull * MAXS
    sizes_arr = np.array(SIZES)
    npieces = nfull + (rem > 0)
    tot = int(npieces.sum())
    p_start = np.empty(tot, dtype=np.int64)
    p_size = np.empty(tot, dtype=np.int64)
    pos = np.cumsum(npieces) - npieces
    # vectorized emit: full pieces
    rep = np.repeat(np.arange(ngroups), nfull)
    k_in = np.arange(len(rep)) - np.repeat(np.cumsum(nfull) - nfull, nfull)
    full_idx = np.repeat(pos, nfull) + k_in
    p_start[full_idx] = m_start[rep] + k_in * MAXS
    p_size[full_idx] = MAXS
    # remainders
    has_rem = rem > 0
    rem_idx = pos[has_rem] + nfull[has_rem]
    p_start[rem_idx] = m_start[has_rem] + nfull[has_rem] * MAXS
    p_size[rem_idx] = rem[has_rem]
    # remainders of 5..7 rows: split 4 + (rem-4) instead of quantizing up
    # to 8 (which would add 1-3 slack rows per piece; 640-768B CCE packets
    # are also disproportionately slow).
    m57 = np.flatnonzero((p_size >= 5) & (p_size <= 7))
    if len(m57):
        a_s = p_start[m57] + 4
        a_l = p_size[m57] - 4
        p_size[m57] = 4
        p_start = np.concatenate([p_start, a_s])
        p_size = np.concatenate([p_size, a_l])
    p_q = sizes_arr[np.searchsorted(sizes_arr, p_size)]
    # Quantization slack past a region's last written row spills into the
    # region's scratch strip (PAD_ROWS) -> always in-bounds, adds zeros.
    return p_start, p_size, p_q


def host_prep(rows, idx, OUT):
    """Dedup + cover + route + pack. Returns (in_maps, geom)."""
    N, _D = rows.shape
    assert _D == D
    SHARD = (OUT + N_CORES - 1) // N_CORES
    n_region = (SHARD + REGION_ROWS - 1) // REGION_ROWS

    inv = np.full(OUT, -1, dtype=np.int64)
    inv[idx] = np.arange(N)  # last occurrence wins
    written = inv >= 0

    p_start, p_size, p_q = _cover_pieces(written, OUT, SHARD, n_region)

    core = p_start // SHARD
    local = p_start - core * SHARD
    region = np.minimum(local // REGION_ROWS, n_region - 1)
    rr = local - region * REGION_ROWS
    parity = rr & 1
    blk = rr >> 1

    # class id = (qsize, parity)
    sizes_arr = np.array(SIZES)
    NCLS = 2 * len(SIZES)

    def group_counts(p_q):
        ci = np.searchsorted(sizes_arr, p_q) * 2 + parity
        key = (core * NCLS + ci) * n_region + region
        order = np.argsort(key, kind="stable")
        key_s = key[order]
        grp_starts = np.concatenate([[0], np.flatnonzero(np.diff(key_s)) + 1])
        grp_keys = key_s[grp_starts]
        grp_cnts = np.diff(np.concatenate([grp_starts, [len(key_s)]]))
        cnt = np.zeros((N_CORES, NCLS, n_region), dtype=np.int64)
        for k, c in zip(grp_keys, grp_cnts):
            r = k % n_region
            cc = k // n_region
            cnt[cc // NCLS, cc % NCLS, r] = c
        return ci, order, grp_starts, grp_keys, grp_cnts, cnt.max(axis=0)

    # consolidate scarce (class, region) groups upward: a group smaller than
    # CONSOLIDATE would spend most of its 128-rounded call on pads, which
    # are real CCE descriptors. Promoting its pieces to the next size only
    # adds slack zeros (spill-safe into the strip).
    for _pass in range(4):
        ci, order, grp_starts, grp_keys, grp_cnts, cnt_max = group_counts(p_q)
        changed = False
        for si in range(len(SIZES) - 1):
            for par in range(2):
                c = si * 2 + par
                for r in range(n_region):
                    n = int(cnt_max[c, r])
                    if 0 < n < CONSOLIDATE:
                        m = (
                            (p_q == SIZES[si])
                            & (parity == par)
                            & (region == r)
                        )
                        p_q[m] = SIZES[si + 1]
                        changed = True
        if not changed:
            break

    # overlap resolution: a piece's quantization/promotion slack must not
    # reach the next piece in the same region (concurrent CCE RMWs on the
    # same bytes race and lose updates). Slack into the scratch strip is
    # fine (zeros racing zeros). Shrink to the exact size when <=8, else
    # split 8 + remainder (all of 1..8 are classes).
    oo = np.argsort(p_start, kind="stable")
    p_start, p_size, p_q = p_start[oo], p_size[oo], p_q[oo]
    rid_all = (p_start // SHARD) * n_region + np.minimum(
        (p_start % SHARD) // REGION_ROWS, n_region - 1
    )
    avail = np.full(len(p_start), 1 << 40, dtype=np.int64)
    same = rid_all[:-1] == rid_all[1:]
    avail[:-1][same] = (p_start[1:] - p_start[:-1])[same]
    over = np.flatnonzero(p_q > avail)
    if len(over):
        add = []
        for i in over:
            L = int(p_size[i])
            if L <= 8:
                p_q[i] = L
                p_size[i] = L
            else:
                p_q[i] = 8
                p_size[i] = 8
                add.append((int(p_start[i]) + 8, L - 8))
        if add:
            a_s = np.array([a[0] for a in add], dtype=np.int64)
            a_l = np.array([a[1] for a in add], dtype=np.int64)
            p_start = np.concatenate([p_start, a_s])
            p_size = np.concatenate([p_size, a_l])
            p_q = np.concatenate([p_q, a_l])

    # recompute derived arrays after the fix-ups
    core = p_start // SHARD
    local = p_start - core * SHARD
    region = np.minimum(local // REGION_ROWS, n_region - 1)
    rr = local - region * REGION_ROWS
    parity = rr & 1
    blk = rr >> 1
    ci, order, grp_starts, grp_keys, grp_cnts, cnt_max = group_counts(p_q)
    grp_counts = grp_cnts

    # chunk geometry: per (ci, region): split count into calls
    chunks = []  # (ci, region, CH, E, base)
    for c in range(NCLS):
        qsize = SIZES[c // 2]
        par = c % 2
        E = qsize * D
        ch_cap = min(CH_CAP, (CAP_ELEMS // E) // 128 * 128)
        for r in range(n_region):
            n = int(cnt_max[c, r])
            if n == 0:
                continue
            nsplit = max(1, -(-n // ch_cap))
            per = -(-n // nsplit)
            per = -(-per // 128) * 128
            left = n
            base = r * SLOT_ROWS * D + par * D
            for _s in range(nsplit):
                take = min(per, max(left, 0))
                CH = max(128, -(-take // 128) * 128)
                chunks.append((c, r, CH, E, base))
                left -= CH
                if left <= 0 and _s + 1 < nsplit:
                    break

    # LPT queue balance by gen-time estimate (~ceil(CH/128) channel blocks)
    costs = [(-(-ch[2] // 128)) * (128 + 10 * ch[3] // 64) for ch in chunks]
    qload = [0] * NQ
    qassign = [0] * len(chunks)
    for t in sorted(range(len(chunks)), key=lambda i: -costs[i]):
        q = min(range(NQ), key=lambda x: qload[x])
        qassign[t] = q
        qload[q] += costs[t]
    # emit interleaved round-robin across queues, each queue big-first
    per_q = [[t for t in sorted(range(len(chunks)), key=lambda i: -costs[i])
              if qassign[t] == q] for q in range(NQ)]
    emit = []
    i = 0
    while any(per_q):
        q = i % NQ
        if per_q[q]:
            emit.append(per_q[q].pop(0))
        i += 1
    chunks = [chunks[t] for t in emit]
    q_of = [qassign[t] for t in emit]

    n_chunks = len(chunks)
    data_elems = sum(ch[2] * ch[3] for ch in chunks)
    idx_cols = sum(ch[2] // 16 for ch in chunks)
    d_offs = [int(x) for x in np.cumsum([0] + [ch[2] * ch[3] for ch in chunks])]
    i_offs = [int(x) for x in np.cumsum([0] + [ch[2] // 16 for ch in chunks])]

    # pack chunks into load groups: one big dma_start per group into a
    # column-ring buffer (per-chunk loads fragment into ~1.5KB descriptors
    # and pay ~30ns/descriptor SDMA overhead).
    groups = []  # (first_chunk, n_in_group, cols)
    col_offs = []  # per chunk: column offset within its group buffer
    g_first, g_cols = 0, 0
    for t, ch in enumerate(chunks):
        SLE = ch[2] * ch[3] // 128
        if g_cols + SLE > GROUP_COLS and t > g_first:
            groups.append((g_first, t - g_first, g_cols))
            g_first, g_cols = t, 0
        col_offs.append(g_cols)
        g_cols += SLE
    groups.append((g_first, n_chunks - g_first, g_cols))

    # --- per-core data/idx packing -------------------------------------
    # per-core, per-(ci, region): ordered piece lists
    in_maps = []
    for cc in range(N_CORES):
        data = np.zeros(data_elems, dtype=ml_dtypes.bfloat16)
        idxw = np.full((16, idx_cols), -1, dtype=np.int16)
        sel_core = core[order] == cc
        # chunk cursor per (ci, region)
        used = {}
        for t, (c, r, CH, E, base) in enumerate(chunks):
            qsize = SIZES[c // 2]
            # pieces for this (core, ci, region)
            kk = (cc * NCLS + c) * n_region + r
            gi = np.searchsorted(grp_keys, kk)
            if gi < len(grp_keys) and grp_keys[gi] == kk:
                g0 = grp_starts[gi]
                gn = grp_counts[gi]
            else:
                g0, gn = 0, 0
            off = used.get((c, r), 0)
            take = max(0, min(gn - off, CH))
            used[(c, r)] = off + take
            # pads target the region's scratch strip: data slots are zeros
            # and +0 races between pads are benign. (Trailing -1 pads would
            # desync the decode-side ring bookkeeping, which sizes the ring
            # from num_idxs_reg while the Q7 trims -> hang.)
            par = c % 2
            nspots = max(1, (PAD_ROWS - par - qsize) // 4 + 1)
            pb_arr = (REGION_ROWS // 2 + 2 * np.arange(nspots)).astype(np.int16)
            it = pb_arr[np.arange(CH) % nspots]
            if take > 0:
                sl = order[g0 + off : g0 + off + take]
                pb = blk[sl].astype(np.int16)
                it[:take] = pb
                # data packing: piece j -> wrap slot, rows [0, qsize)
                SL = CH // 128
                j = np.arange(take)
                wrap = (j % 128) * SL + j // 128
                view = data[d_offs[t] : d_offs[t] + CH * E].reshape(
                    CH * qsize, D
                )
                # source rows: global start + k, valid if k < size and written
                g_start = p_start[sl]
                g_size = p_size[sl]
                k = np.arange(qsize)
                rowg = g_start[:, None] + k[None, :]
                valid = (k[None, :] < g_size[:, None]) & (
                    inv[np.minimum(rowg, OUT - 1)] >= 0
                )
                srcrow = inv[np.minimum(rowg, OUT - 1)]
                dst = wrap[:, None] * qsize + k[None, :]
                vm = valid.ravel()
                view[dst.ravel()[vm]] = rows[srcrow.ravel()[vm]]
            iw = it.reshape(CH // 16, 16).T  # [16, CW]
            idxw[:, i_offs[t] : i_offs[t + 1]] = iw
        # rearrange chunk-major blob into group-major [128, cols] layout to
        # match the grouped load AP (partition p <- blob[p * group_cols])
        nd = np.empty_like(data)
        for (first, gn, cols) in groups:
            off = d_offs[first]
            gblks = [
                data[d_offs[t] : d_offs[t] + chunks[t][2] * chunks[t][3]].reshape(
                    128, chunks[t][2] * chunks[t][3] // 128
                )
                for t in range(first, first + gn)
            ]
            nd[off : off + 128 * cols] = np.concatenate(gblks, axis=1).reshape(-1)
        data = nd
        iwf = np.ascontiguousarray(
            np.broadcast_to(idxw[None], (8, 16, idx_cols))
        ).reshape(128, idx_cols)
        in_maps.append({"rows": data.reshape(-1, D), "idxw": iwf})

    shard_alloc = n_region * SLOT_ROWS
    geom = (
        tuple(chunks),
        tuple(q_of),
        tuple(groups),
        tuple(col_offs),
        shard_alloc,
        SHARD,
        data_elems,
        idx_cols,
    )
    return in_maps, geom


_prog_cache = {}


def build_program(geom, repeats=1):
    chunks, q_of, groups, col_offs, shard_alloc, SHARD, data_elems, idx_cols = geom
    key = (chunks, q_of, groups, shard_alloc, repeats)
    if key in _prog_cache:
        return _prog_cache[key]
    nc = bacc.Bacc(None, num_swdge_queues=NQ)
    rows_t = nc.dram_tensor(
        "rows", [data_elems // D, D], mybir.dt.bfloat16, kind="ExternalInput"
    )
    idxw_t = nc.dram_tensor(
        "idxw", [128, idx_cols], mybir.dt.int16, kind="ExternalInput"
    )
    out_t = nc.dram_tensor(
        "out", [shard_alloc, D], mybir.dt.bfloat16, kind="ExternalOutput"
    )

    n_chunks = len(chunks)
    n_groups = len(groups)
    d_offs = [int(x) for x in np.cumsum([0] + [ch[2] * ch[3] for ch in chunks])]
    i_offs = [int(x) for x in np.cumsum([0] + [ch[2] // 16 for ch in chunks])]

    qseq = []
    qcnt = [0] * NQ
    for tc in range(n_chunks):
        qcnt[q_of[tc]] += 1
        qseq.append(qcnt[q_of[tc]])
    qtot = list(qcnt)
    # per-group: cumulative chunks per queue through the END of group gi
    chunk_group = [0] * n_chunks
    qcum = []
    run = [0] * NQ
    for gi, (first, gn, _cols) in enumerate(groups):
        for tc in range(first, first + gn):
            chunk_group[tc] = gi
            run[q_of[tc]] += 1
        qcum.append(list(run))

    with (
        nc.semaphore("idx_sem") as idx_sem,
        nc.semaphore("sq0") as sq0,
        nc.semaphore("sq1") as sq1,
        nc.semaphore("sq2") as sq2,
        nc.semaphore("sq3") as sq3,
    ):
        scat_sems = [sq0, sq1, sq2, sq3]
        # One load-completion semaphore per data buffer. A single shared
        # counter is ambiguous under SDMA engine skew (engines drain their
        # FIFO rings at different rates, so 14 engines can be a whole load
        # ahead while 2 lag and the aggregate count still reaches the
        # threshold with load t incomplete). Per-buffer counters are exact:
        # buffer reuse is gated on the consuming scatter, so sem_b counts
        # strictly ordered loads.
        bsems = [
            nc.ctx.enter_context(nc.semaphore(f"ld{b}")) for b in range(NBG)
        ]
        data_sb = [
            nc.ctx.enter_context(
                nc.sbuf_tensor(f"data{b}", [128, GROUP_COLS], mybir.dt.bfloat16)
            )
            for b in range(NBG)
        ]
        idx_sb = nc.ctx.enter_context(
            nc.sbuf_tensor("idxs", [128, idx_cols], mybir.dt.int16)
        )

        with nc.Block() as block:

            @block.sync
            def _(sync):
                sync.dma_start(
                    out=AP(idx_sb, 0, [[idx_cols, 128], [1, idx_cols]]),
                    in_=AP(idxw_t, 0, [[idx_cols, 128], [1, idx_cols]]),
                ).then_inc(idx_sem, 16)
                G = 0
                for _r in range(repeats):
                    for gi, (first, gn, cols) in enumerate(groups):
                        b = G % NBG
                        if G >= NBG:
                            Gp = G - NBG
                            rp, gp = Gp // n_groups, Gp % n_groups
                            for q in range(NQ):
                                v = rp * qtot[q] + qcum[gp][q]
                                if v > 0:
                                    sync.wait_ge(scat_sems[q], 16 * v)
                        sync.dma_start(
                            out=AP(data_sb[b], 0, [[GROUP_COLS, 128], [1, cols]]),
                            in_=AP(rows_t, d_offs[first], [[cols, 128], [1, cols]]),
                        ).then_inc(bsems[b], 16)
                        G += 1

            @block.gpsimd
            def _(g):
                t = 0
                for _r in range(repeats):
                    for tc in range(n_chunks):
                        _, _, CH, E, base = chunks[tc]
                        SL = CH // 128
                        q = q_of[tc]
                        gi = chunk_group[tc]
                        G = _r * n_groups + gi
                        b = G % NBG
                        if t == 0:
                            g.wait_ge(idx_sem, 16)
                        if tc == groups[gi][0]:
                            g.wait_ge(bsems[b], 16 * (G // NBG + 1))
                        par_off = base % (BSTEP * D)
                        nblk = (SLOT_ROWS * D - par_off - E) // (BSTEP * D) + 1
                        g.dma_scatter_add(
                            AP(
                                out_t,
                                base,
                                [[BSTEP * D, nblk], [1, E]],
                            ),
                            AP(
                                data_sb[b],
                                col_offs[tc],
                                [[GROUP_COLS, 128], [E, SL], [1, E]],
                            ),
                            AP(idx_sb, i_offs[tc], [[idx_cols, 128], [1, CH // 16]]),
                            CH,
                            CH,
                            E,
                            elem_step=BSTEP * D,
                            queue_num=q,
                        ).then_inc(scat_sems[q], 16)
                        t += 1
                for q in range(NQ):
                    g.wait_ge(scat_sems[q], 16 * repeats * qtot[q])

    nc.finalize()
    _prog_cache[key] = nc
    return nc


LAST_PREP = None


def kernel(input_, indices, output_size, n_tpc):
    global LAST_PREP
    rows = np.asarray(input_)
    in_dtype = rows.dtype
    if rows.dtype != ml_dtypes.bfloat16:
        rows = rows.astype(ml_dtypes.bfloat16)
    idx = np.asarray(indices).astype(np.int64)
    OUT = int(output_size)

    in_maps, geom = host_prep(rows, idx, OUT)
    LAST_PREP = (in_maps, geom)
    SHARD = geom[5]
    nc = build_program(geom)
    res = bass_utils.run_bass_kernel_spmd(nc, in_maps, core_ids=list(range(N_CORES)))

    n_region = geom[4] // SLOT_ROWS
    out_full = np.concatenate(
        [
            r["out"]
            .reshape(n_region, SLOT_ROWS, D)[:, :REGION_ROWS]
            .reshape(-1, D)[:SHARD]
            for r in res.results
        ],
        axis=0,
    )[:OUT]
    return np.ascontiguousarray(out_full.astype(in_dtype))


# revision 47
# speedup vs baseline: 1.0340x; 1.0340x over previous
"""Trainium2 Bass kernel: scatter rows of input_ into a zero-initialized
[output_size, D] bf16 buffer: out[indices[i], :] = input_[i, :] (last
occurrence wins for duplicate indices).

Strategy (8 NeuronCores), measured at ~330us HW vs the 426us fixed-class
baseline (both bit-exact):
  - Output row-sharded by index range: core k owns rows [k*SHARD, (k+1)*SHARD).
  - Each core's shard is split into 65280-row regions, each followed by a
    128-row scratch strip in the out tensor (idx is an int16 2-row-block
    number, so a region maxes out at ~65408 rows; the strip absorbs
    quantization slack and hosts pad-descriptor targets).
  - Host routing: dedup indices last-wins; cover the written rows with
    dma_scatter_add descriptors: maximal written-row runs are merged across
    gaps of <=GAP unwritten rows (bridged rows are zero-filled in the packed
    data; CCE add of 0 is a no-op), then split into pieces of 1..4 rows.
    Keeping every CCE packet <=512B matters: measured per-packet SDMA engine
    time is ~28.5ns + ~0.072ns/B up to 512B but grows superlinearly past
    that (640-768B packets cost ~0.1ns/B), so size classes 5+ lose.
  - An overlap-resolution pass shrinks/splits any piece whose quantization
    slack would reach the next piece: concurrent CCE read-modify-writes on
    shared bytes race across queues and lose updates.
  - Pieces are grouped into (size, start-parity) classes per region; scarce
    (class, region) groups are promoted upward (CONSOLIDATE) so calls don't
    drown in 128-rounding pads. Pad slots carry valid targets spread over
    the scratch strip with all-zero data (+0 races with other pads are
    benign). Trailing -1 pads would desync the decode-side ring bookkeeping,
    which sizes the ring from num_idxs_reg while the Q7 trims -> hang.
  - Packed data is loaded in ~5MB groups into a 4-buffer column ring (one
    dma_start per group; per-chunk loads fragment into ~1.5KB descriptors
    that pay ~30ns each). One load-completion semaphore PER BUFFER: a single
    shared counter is ambiguous under SDMA engine skew (15 engines can run a
    whole load ahead while one lags, satisfying the aggregate threshold
    while that load is incomplete on the straggler -> stale-data scatters).
  - Chunks are LPT-balanced across the 4 SWDGE queues (Q7 core pairs) by
    estimated descriptor-generation time and emitted round-robin so all 4
    queue pairs generate concurrently.
  - The output is donated pre-zeroed by run_bass_kernel_spmd / bass2jax, so
    CCE add == set (every written row is added exactly once onto zeros).
"""

import os
import sys

sys.path.insert(0, "/opt/trn_rl_repo")
os.environ.setdefault("JAX_PLATFORMS", "axon")

import numpy as np
import ml_dtypes

from concourse import bacc, mybir
from concourse.bass import AP
from concourse import bass_utils

N_CORES = 8
REGION_ROWS = 65280  # idx = 2-row block number: 32640 blocks, int16-safe
PAD_ROWS = 128  # scratch strip after each region: pad targets + slack spill
SLOT_ROWS = REGION_ROWS + PAD_ROWS  # region slot stride in the out tensor
GAP = 2  # merge written-row runs separated by <= GAP unwritten rows
MAXS = 4  # max descriptor size in rows
SIZES = (1, 2, 3, 4)  # class sizes (rows)
CH_CAP = 7936  # per-call index cap: tx ring needs 2*CH/16+1 < 1024 descs
CAP_ELEMS = 786432  # per-call data tile cap (CH * E elems, 6144 SBUF cols)
NQ = 4  # SWDGE queues == Q7 core pairs generating descriptors in parallel
NBG = 4  # SBUF load-group ring buffers
GROUP_COLS = 20480  # columns per load-group buffer (40 KiB / partition)
CONSOLIDATE = 96  # re-quantize (class, region) groups smaller than this

D = 64
BSTEP = 2  # descriptor address step: 2 rows = 256B, idx in 2-row blocks


def _runs_of(mask):
    d = np.diff(np.concatenate([[0], mask.view(np.int8), [0]]))
    return np.flatnonzero(d == 1), np.flatnonzero(d == -1)  # starts, ends(excl)


def _cover_pieces(written, OUT, SHARD, n_region):
    """Global covering: returns (starts, sizes, qsizes) of descriptor pieces.
    Runs are split at shard-local region boundaries, gap-merged, split to
    <=MAXS, and remainders quantized up to the class set. Quantization slack
    may spill into the region's scratch strip, which is harmless."""

    def rid(g):
        cc = g // SHARD
        return cc * n_region + np.minimum((g - cc * SHARD) // REGION_ROWS,
                                          n_region - 1)

    starts, ends = _runs_of(written)
    # split runs crossing region boundaries
    cross = np.flatnonzero(rid(starts) != rid(ends - 1))
    if len(cross):
        add_s, add_e = [], []
        for i in cross:
            s, e = int(starts[i]), int(ends[i])
            cuts = set()
            for cc in range(s // SHARD, e // SHARD + 1):
                if s < cc * SHARD < e:
                    cuts.add(cc * SHARD)
                for r in range(1, n_region):
                    b = cc * SHARD + r * REGION_ROWS
                    if s < b < e:
                        cuts.add(b)
            seq = np.array([s] + sorted(cuts) + [e])
            add_s.append(seq[:-1])
            add_e.append(seq[1:])
        keep = np.ones(len(starts), bool)
        keep[cross] = False
        starts = np.concatenate([starts[keep]] + add_s)
        ends = np.concatenate([ends[keep]] + add_e)
        o = np.argsort(starts)
        starts, ends = starts[o], ends[o]
    # gap-merge within regions
    gap_ok = (starts[1:] - ends[:-1]) <= GAP
    same_reg = rid(starts[1:]) == rid(ends[:-1] - 1)
    brk = np.concatenate([[True], ~(gap_ok & same_reg)])
    gid = np.cumsum(brk) - 1
    ngroups = gid[-1] + 1
    m_start = np.full(ngroups, OUT, dtype=np.int64)
    np.minimum.at(m_start, gid, starts)
    m_end = np.zeros(ngroups, dtype=np.int64)
    np.maximum.at(m_end, gid, ends)
    L = m_end - m_start
    # split into pieces of MAXS + quantized remainder
    nfull = L // MAXS
    rem = L - nfull * MAXS
    sizes_arr = np.array(SIZES)
    npieces = nfull + (rem > 0)
    tot = int(npieces.sum())
    p_start = np.empty(tot, dtype=np.int64)
    p_size = np.empty(tot, dtype=np.int64)
    pos = np.cumsum(npieces) - npieces
    # vectorized emit: full pieces
    rep = np.repeat(np.arange(ngroups), nfull)
    k_in = np.arange(len(rep)) - np.repeat(np.cumsum(nfull) - nfull, nfull)
    full_idx = np.repeat(pos, nfull) + k_in
    p_start[full_idx] = m_start[rep] + k_in * MAXS
    p_size[full_idx] = MAXS
    # remainders
    has_rem = rem > 0
    rem_idx = pos[has_rem] + nfull[has_rem]
    p_start[rem_idx] = m_start[has_rem] + nfull[has_rem] * MAXS
    p_size[rem_idx] = rem[has_rem]
    p_q = sizes_arr[np.searchsorted(sizes_arr, p_size)]
    # Quantization slack past a region's last written row spills into the
    # region's scratch strip (PAD_ROWS) -> always in-bounds, adds zeros.
    return p_start, p_size, p_q


def host_prep(rows, idx, OUT):
    """Dedup + cover + route + pack. Returns (in_maps, geom)."""
    N, _D = rows.shape
    assert _D == D
    SHARD = (OUT + N_CORES - 1) // N_CORES
    n_region = (SHARD + REGION_ROWS - 1) // REGION_ROWS

    inv = np.full(OUT, -1, dtype=np.int64)
    inv[idx] = np.arange(N)  # last occurrence wins
    written = inv >= 0

    p_start, p_size, p_q = _cover_pieces(written, OUT, SHARD, n_region)

    core = p_start // SHARD
    local = p_start - core * SHARD
    region = np.minimum(local // REGION_ROWS, n_region - 1)
    rr = local - region * REGION_ROWS
    parity = rr & 1
    blk = rr >> 1

    # class id = (qsize, parity)
    sizes_arr = np.array(SIZES)
    NCLS = 2 * len(SIZES)

    def group_counts(p_q):
        ci = np.searchsorted(sizes_arr, p_q) * 2 + parity
        key = (core * NCLS + ci) * n_region + region
        order = np.argsort(key, kind="stable")
        key_s = key[order]
        grp_starts = np.concatenate([[0], np.flatnonzero(np.diff(key_s)) + 1])
        grp_keys = key_s[grp_starts]
        grp_cnts = np.diff(np.concatenate([grp_starts, [len(key_s)]]))
        cnt = np.zeros((N_CORES, NCLS, n_region), dtype=np.int64)
        for k, c in zip(grp_keys, grp_cnts):
            r = k % n_region
            cc = k // n_region
            cnt[cc // NCLS, cc % NCLS, r] = c
        return ci, order, grp_starts, grp_keys, grp_cnts, cnt.max(axis=0)

    # consolidate scarce (class, region) groups upward: a group smaller than
    # CONSOLIDATE would spend most of its 128-rounded call on pads, which
    # are real CCE descriptors. Promoting its pieces to the next size only
    # adds slack zeros (spill-safe into the strip).
    for _pass in range(4):
        ci, order, grp_starts, grp_keys, grp_cnts, cnt_max = group_counts(p_q)
        changed = False
        for si in range(len(SIZES) - 1):
            for par in range(2):
                c = si * 2 + par
                for r in range(n_region):
                    n = int(cnt_max[c, r])
                    if 0 < n < CONSOLIDATE:
                        m = (
                            (p_q == SIZES[si])
                            & (parity == par)
                            & (region == r)
                        )
                        p_q[m] = SIZES[si + 1]
                        changed = True
        if not changed:
            break

    # overlap resolution: a piece's quantization/promotion slack must not
    # reach the next piece in the same region (concurrent CCE RMWs on the
    # same bytes race and lose updates). Slack into the scratch strip is
    # fine (zeros racing zeros). Shrink to the exact size when <=8, else
    # split 8 + remainder (all of 1..8 are classes).
    oo = np.argsort(p_start, kind="stable")
    p_start, p_size, p_q = p_start[oo], p_size[oo], p_q[oo]
    rid_all = (p_start // SHARD) * n_region + np.minimum(
        (p_start % SHARD) // REGION_ROWS, n_region - 1
    )
    avail = np.full(len(p_start), 1 << 40, dtype=np.int64)
    same = rid_all[:-1] == rid_all[1:]
    avail[:-1][same] = (p_start[1:] - p_start[:-1])[same]
    over = np.flatnonzero(p_q > avail)
    if len(over):
        add = []
        for i in over:
            L = int(p_size[i])
            if L <= 8:
                p_q[i] = L
                p_size[i] = L
            else:
                p_q[i] = 8
                p_size[i] = 8
                add.append((int(p_start[i]) + 8, L - 8))
        if add:
            a_s = np.array([a[0] for a in add], dtype=np.int64)
            a_l = np.array([a[1] for a in add], dtype=np.int64)
            p_start = np.concatenate([p_start, a_s])
            p_size = np.concatenate([p_size, a_l])
            p_q = np.concatenate([p_q, a_l])

    # recompute derived arrays after the fix-ups
    core = p_start // SHARD
    local = p_start - core * SHARD
    region = np.minimum(local // REGION_ROWS, n_region - 1)
    rr = local - region * REGION_ROWS
    parity = rr & 1
    blk = rr >> 1
    ci, order, grp_starts, grp_keys, grp_cnts, cnt_max = group_counts(p_q)
    grp_counts = grp_cnts

    # chunk geometry: per (ci, region): split count into calls
    chunks = []  # (ci, region, CH, E, base)
    for c in range(NCLS):
        qsize = SIZES[c // 2]
        par = c % 2
        E = qsize * D
        ch_cap = min(CH_CAP, (CAP_ELEMS // E) // 128 * 128)
        for r in range(n_region):
            n = int(cnt_max[c, r])
            if n == 0:
                continue
            nsplit = max(1, -(-n // ch_cap))
            per = -(-n // nsplit)
            per = -(-per // 128) * 128
            left = n
            base = r * SLOT_ROWS * D + par * D
            for _s in range(nsplit):
                take = min(per, max(left, 0))
                CH = max(128, -(-take // 128) * 128)
                chunks.append((c, r, CH, E, base))
                left -= CH
                if left <= 0 and _s + 1 < nsplit:
                    break

    # LPT queue balance by gen-time estimate (~ceil(CH/128) channel blocks)
    costs = [(-(-ch[2] // 128)) * (128 + 10 * ch[3] // 64) for ch in chunks]
    qload = [0] * NQ
    qassign = [0] * len(chunks)
    for t in sorted(range(len(chunks)), key=lambda i: -costs[i]):
        q = min(range(NQ), key=lambda x: qload[x])
        qassign[t] = q
        qload[q] += costs[t]
    # emit interleaved round-robin across queues, each queue big-first
    per_q = [[t for t in sorted(range(len(chunks)), key=lambda i: -costs[i])
              if qassign[t] == q] for q in range(NQ)]
    emit = []
    i = 0
    while any(per_q):
        q = i % NQ
        if per_q[q]:
            emit.append(per_q[q].pop(0))
        i += 1
    chunks = [chunks[t] for t in emit]
    q_of = [qassign[t] for t in emit]

    n_chunks = len(chunks)
    data_elems = sum(ch[2] * ch[3] for ch in chunks)
    idx_cols = sum(ch[2] // 16 for ch in chunks)
    d_offs = [int(x) for x in np.cumsum([0] + [ch[2] * ch[3] for ch in chunks])]
    i_offs = [int(x) for x in np.cumsum([0] + [ch[2] // 16 for ch in chunks])]

    # pack chunks into load groups: one big dma_start per group into a
    # column-ring buffer (per-chunk loads fragment into ~1.5KB descriptors
    # and pay ~30ns/descriptor SDMA overhead).
    groups = []  # (first_chunk, n_in_group, cols)
    col_offs = []  # per chunk: column offset within its group buffer
    g_first, g_cols = 0, 0
    for t, ch in enumerate(chunks):
        SLE = ch[2] * ch[3] // 128
        if g_cols + SLE > GROUP_COLS and t > g_first:
            groups.append((g_first, t - g_first, g_cols))
            g_first, g_cols = t, 0
        col_offs.append(g_cols)
        g_cols += SLE
    groups.append((g_first, n_chunks - g_first, g_cols))

    # --- per-core data/idx packing -------------------------------------
    # per-core, per-(ci, region): ordered piece lists
    in_maps = []
    for cc in range(N_CORES):
        data = np.zeros(data_elems, dtype=ml_dtypes.bfloat16)
        idxw = np.full((16, idx_cols), -1, dtype=np.int16)
        sel_core = core[order] == cc
        # chunk cursor per (ci, region)
        used = {}
        for t, (c, r, CH, E, base) in enumerate(chunks):
            qsize = SIZES[c // 2]
            # pieces for this (core, ci, region)
            kk = (cc * NCLS + c) * n_region + r
            gi = np.searchsorted(grp_keys, kk)
            if gi < len(grp_keys) and grp_keys[gi] == kk:
                g0 = grp_starts[gi]
                gn = grp_counts[gi]
            else:
                g0, gn = 0, 0
            off = used.get((c, r), 0)
            take = max(0, min(gn - off, CH))
            used[(c, r)] = off + take
            # pads target the region's scratch strip: data slots are zeros
            # and +0 races between pads are benign. (Trailing -1 pads would
            # desync the decode-side ring bookkeeping, which sizes the ring
            # from num_idxs_reg while the Q7 trims -> hang.)
            par = c % 2
            nspots = max(1, (PAD_ROWS - par - qsize) // 4 + 1)
            pb_arr = (REGION_ROWS // 2 + 2 * np.arange(nspots)).astype(np.int16)
            it = pb_arr[np.arange(CH) % nspots]
            if take > 0:
                sl = order[g0 + off : g0 + off + take]
                pb = blk[sl].astype(np.int16)
                it[:take] = pb
                # data packing: piece j -> wrap slot, rows [0, qsize)
                SL = CH // 128
                j = np.arange(take)
                wrap = (j % 128) * SL + j // 128
                view = data[d_offs[t] : d_offs[t] + CH * E].reshape(
                    CH * qsize, D
                )
                # source rows: global start + k, valid if k < size and written
                g_start = p_start[sl]
                g_size = p_size[sl]
                k = np.arange(qsize)
                rowg = g_start[:, None] + k[None, :]
                valid = (k[None, :] < g_size[:, None]) & (
                    inv[np.minimum(rowg, OUT - 1)] >= 0
                )
                srcrow = inv[np.minimum(rowg, OUT - 1)]
                dst = wrap[:, None] * qsize + k[None, :]
                vm = valid.ravel()
                view[dst.ravel()[vm]] = rows[srcrow.ravel()[vm]]
            iw = it.reshape(CH // 16, 16).T  # [16, CW]
            idxw[:, i_offs[t] : i_offs[t + 1]] = iw
        # rearrange chunk-major blob into group-major [128, cols] layout to
        # match the grouped load AP (partition p <- blob[p * group_cols])
        nd = np.empty_like(data)
        for (first, gn, cols) in groups:
            off = d_offs[first]
            gblks = [
                data[d_offs[t] : d_offs[t] + chunks[t][2] * chunks[t][3]].reshape(
                    128, chunks[t][2] * chunks[t][3] // 128
                )
                for t in range(first, first + gn)
            ]
            nd[off : off + 128 * cols] = np.concatenate(gblks, axis=1).reshape(-1)
        data = nd
        iwf = np.ascontiguousarray(
            np.broadcast_to(idxw[None], (8, 16, idx_cols))
        ).reshape(128, idx_cols)
        in_maps.append({"rows": data.reshape(-1, D), "idxw": iwf})

    shard_alloc = n_region * SLOT_ROWS
    geom = (
        tuple(chunks),
        tuple(q_of),
        tuple(groups),
        tuple(col_offs),
        shard_alloc,
        SHARD,
        data_elems,
        idx_cols,
    )
    return in_maps, geom


_prog_cache = {}


def build_program(geom, repeats=1):
    chunks, q_of, groups, col_offs, shard_alloc, SHARD, data_elems, idx_cols = geom
    key = (chunks, q_of, groups, shard_alloc, repeats)
    if key in _prog_cache:
        return _prog_cache[key]
    nc = bacc.Bacc(None, num_swdge_queues=NQ)
    rows_t = nc.dram_tensor(
        "rows", [data_elems // D, D], mybir.dt.bfloat16, kind="ExternalInput"
    )
    idxw_t = nc.dram_tensor(
        "idxw", [128, idx_cols], mybir.dt.int16, kind="ExternalInput"
    )
    out_t = nc.dram_tensor(
        "out", [shard_alloc, D], mybir.dt.bfloat16, kind="ExternalOutput"
    )

    n_chunks = len(chunks)
    n_groups = len(groups)
    d_offs = [int(x) for x in np.cumsum([0] + [ch[2] * ch[3] for ch in chunks])]
    i_offs = [int(x) for x in np.cumsum([0] + [ch[2] // 16 for ch in chunks])]

    qseq = []
    qcnt = [0] * NQ
    for tc in range(n_chunks):
        qcnt[q_of[tc]] += 1
        qseq.append(qcnt[q_of[tc]])
    qtot = list(qcnt)
    # per-group: cumulative chunks per queue through the END of group gi
    chunk_group = [0] * n_chunks
    qcum = []
    run = [0] * NQ
    for gi, (first, gn, _cols) in enumerate(groups):
        for tc in range(first, first + gn):
            chunk_group[tc] = gi
            run[q_of[tc]] += 1
        qcum.append(list(run))

    with (
        nc.semaphore("idx_sem") as idx_sem,
        nc.semaphore("sq0") as sq0,
        nc.semaphore("sq1") as sq1,
        nc.semaphore("sq2") as sq2,
        nc.semaphore("sq3") as sq3,
    ):
        scat_sems = [sq0, sq1, sq2, sq3]
        # One load-completion semaphore per data buffer. A single shared
        # counter is ambiguous under SDMA engine skew (engines drain their
        # FIFO rings at different rates, so 14 engines can be a whole load
        # ahead while 2 lag and the aggregate count still reaches the
        # threshold with load t incomplete). Per-buffer counters are exact:
        # buffer reuse is gated on the consuming scatter, so sem_b counts
        # strictly ordered loads.
        bsems = [
            nc.ctx.enter_context(nc.semaphore(f"ld{b}")) for b in range(NBG)
        ]
        data_sb = [
            nc.ctx.enter_context(
                nc.sbuf_tensor(f"data{b}", [128, GROUP_COLS], mybir.dt.bfloat16)
            )
            for b in range(NBG)
        ]
        idx_sb = nc.ctx.enter_context(
            nc.sbuf_tensor("idxs", [128, idx_cols], mybir.dt.int16)
        )

        with nc.Block() as block:

            @block.sync
            def _(sync):
                sync.dma_start(
                    out=AP(idx_sb, 0, [[idx_cols, 128], [1, idx_cols]]),
                    in_=AP(idxw_t, 0, [[idx_cols, 128], [1, idx_cols]]),
                ).then_inc(idx_sem, 16)
                G = 0
                for _r in range(repeats):
                    for gi, (first, gn, cols) in enumerate(groups):
                        b = G % NBG
                        if G >= NBG:
                            Gp = G - NBG
                            rp, gp = Gp // n_groups, Gp % n_groups
                            for q in range(NQ):
                                v = rp * qtot[q] + qcum[gp][q]
                                if v > 0:
                                    sync.wait_ge(scat_sems[q], 16 * v)
                        sync.dma_start(
                            out=AP(data_sb[b], 0, [[GROUP_COLS, 128], [1, cols]]),
                            in_=AP(rows_t, d_offs[first], [[cols, 128], [1, cols]]),
                        ).then_inc(bsems[b], 16)
                        G += 1

            @block.gpsimd
            def _(g):
                t = 0
                for _r in range(repeats):
                    for tc in range(n_chunks):
                        _, _, CH, E, base = chunks[tc]
                        SL = CH // 128
                        q = q_of[tc]
                        gi = chunk_group[tc]
                        G = _r * n_groups + gi
                        b = G % NBG
                        if t == 0:
                            g.wait_ge(idx_sem, 16)
                        if tc == groups[gi][0]:
                            g.wait_ge(bsems[b], 16 * (G // NBG + 1))
                        par_off = base % (BSTEP * D)
                        nblk = (SLOT_ROWS * D - par_off - E) // (BSTEP * D) + 1
                        g.dma_scatter_add(
                            AP(
                                out_t,
                                base,
                                [[BSTEP * D, nblk], [1, E]],
                            ),
                            AP(
                                data_sb[b],
                                col_offs[tc],
                                [[GROUP_COLS, 128], [E, SL], [1, E]],
                            ),
                            AP(idx_sb, i_offs[tc], [[idx_cols, 128], [1, CH // 16]]),
                            CH,
                            CH,
                            E,
                            elem_step=BSTEP * D,
                            queue_num=q,
                        ).then_inc(scat_sems[q], 16)
                        t += 1
                for q in range(NQ):
                    g.wait_ge(scat_sems[q], 16 * repeats * qtot[q])

    nc.finalize()
    _prog_cache[key] = nc
    return nc


LAST_PREP = None


def kernel(input_, indices, output_size, n_tpc):
    global LAST_PREP
    rows = np.asarray(input_)
    in_dtype = rows.dtype
    if rows.dtype != ml_dtypes.bfloat16:
        rows = rows.astype(ml_dtypes.bfloat16)
    idx = np.asarray(indices).astype(np.int64)
    OUT = int(output_size)

    in_maps, geom = host_prep(rows, idx, OUT)
    LAST_PREP = (in_maps, geom)
    SHARD = geom[5]
    nc = build_program(geom)
    res = bass_utils.run_bass_kernel_spmd(nc, in_maps, core_ids=list(range(N_CORES)))

    n_region = geom[4] // SLOT_ROWS
    out_full = np.concatenate(
        [
            r["out"]
            .reshape(n_region, SLOT_ROWS, D)[:, :REGION_ROWS]
            .reshape(-1, D)[:SHARD]
            for r in res.results
        ],
        axis=0,
    )[:OUT]
    return np.ascontiguousarray(out_full.astype(in_dtype))
